# revision 10
# baseline (speedup 1.0000x reference)
"""Multi-head attention (S=2048, D=1024, H=16, dk=dv=64) on 8 TRN2 NeuronCores.

Sharding: head-parallel tensor parallelism. Core c owns heads {2c, 2c+1}:
  - QT/KT [128, S] (two heads stacked on partitions); V via PE-transpose of
    VT, augmented with a ones column so the ctx matmul also produces the
    softmax denominators (softmax runs over the partition axis).
  - scoresT tiles -> exp on ACT (scale=1/8 folded in) -> ctx accumulation.
  - per s-chunk: normalize ctxT, AllGather the [128, chunk] block across
    cores -> [1024, chunk] concat, then a 128-column slice of the output
    projection per core (outT layout). Host unshard = concat + transpose.

Overlap structure: K and the first half of Q are projected first (minimum
needed to start the exp stream); enc_v and the second Q half stream in
DURING the first chunk's scores/exp, with their projections accumulated in
SBUF via transient PSUM slots, so the ACT engine (the bottleneck) runs
gaplessly from ~1/3 into the kernel to the end.

Compute dtype: bf16 operands, fp32 PSUM accumulation, softmax in fp32.
"""

import numpy as np

import concourse.bass as bass
import concourse.mybir as mybir
import concourse.tile as tile
from concourse import bacc
from concourse.bass_utils import run_bass_kernel_spmd

S = 2048
D = 1024
H = 16
DK = 64
DV = 64
NCORES = 8
HPC = H // NCORES          # heads per core = 2
FW = HPC * DV              # per-core feature width = 128
P = 128                    # partitions
KT_D = D // P              # 8 contraction tiles over D
TT = S // P                # 16 tiles over t (keys)
SCH = 1024                 # s-chunk width for attention
NSC = S // SCH             # 2 s-chunks
NQ = 512                   # matmul moving free dim
VA = 2 * (DV + 1)          # V_aug feature width (2 heads x (64 + ones))

F32 = mybir.dt.float32
BF16 = mybir.dt.bfloat16
EXPF = mybir.ActivationFunctionType.Exp

_cache = {}


def build():
    nc = bacc.Bacc(None, target_bir_lowering=False)

    enc_in = {
        x: nc.dram_tensor(f"enc{x}_t", [D, S], F32, kind="ExternalInput")
        for x in ("q", "k", "v")
    }
    w_in = {
        n: nc.dram_tensor(n, [D, FW], F32, kind="ExternalInput")
        for n in ("wq", "wk", "wv", "wo")
    }
    out_t = nc.dram_tensor("outT", [FW, S], F32, kind="ExternalOutput")

    with tile.TileContext(nc) as tc:
        with (
            tc.tile_pool(name="wts", bufs=1) as wts,
            tc.tile_pool(name="encp", bufs=2) as encp,
            tc.tile_pool(name="qkv", bufs=1) as qkv,
            tc.tile_pool(name="expp", bufs=16) as expp,
            tc.tile_pool(name="catp", bufs=1) as catp,
            tc.tile_pool(name="catin", bufs=3) as catin,
            tc.tile_pool(name="misc", bufs=2) as misc,
            tc.tile_pool(name="dram", bufs=1, space="DRAM") as dram,
        ):
            rg = [list(range(NCORES))]

            # ---- weights: cast-DMA f32 -> bf16, [128, KT_D, FW] ----
            wtiles = {}
            for name in ("wq", "wk", "wv", "wo"):
                wt = wts.tile([P, KT_D, FW], BF16, tag=f"w_{name}", name=name)
                nc.gpsimd.dma_start(
                    wt[:], w_in[name].rearrange("(kt p) m -> p kt m", p=P)
                )
                wtiles[name] = wt

            ident = wts.tile([P, P], BF16, tag="ident")
            from concourse.masks import make_identity

            make_identity(nc, ident)

            # persistent SBUF state
            qt_sb = qkv.tile([P, S], BF16, tag="qt")
            kt_sb = qkv.tile([P, S], BF16, tag="kt")
            vt_sb = qkv.tile([P, S], BF16, tag="vt")
            v_aug = qkv.tile([P, TT, VA], BF16, tag="vaug")
            vt_acc = qkv.tile([P, S], F32, tag="vtacc")
            qt_acc = qkv.tile([P, SCH], F32, tag="qtacc")
            cat_loc = catp.tile([P, S], BF16, tag="cat")
            out_sb = catp.tile([P, S], F32, tag="outsb")
            nc.any.memset(v_aug[:, :, DV : DV + 1], 1.0)
            nc.any.memset(v_aug[:, :, 2 * DV + 1 : 2 * DV + 2], 1.0)

            def load_enc(x, dt, cast_eng, cols=None, tagsuf=""):
                c0, c1 = (0, S) if cols is None else cols
                raw = encp.tile(
                    [P, c1 - c0], F32, tag=f"raw{tagsuf}", name="raw"
                )
                nc.sync.dma_start(
                    raw[:], enc_in[x][dt * P : (dt + 1) * P, c0:c1]
                )
                t = encp.tile([P, c1 - c0], BF16, tag=f"bf{tagsuf}", name="bf")
                if cast_eng == "act":
                    nc.scalar.copy(t[:], raw[:])
                else:
                    nc.vector.tensor_copy(t[:], raw[:])
                return t

            # ---- phase 0: K full projection, Q first half ----
            ps_p_cm = tc.tile_pool(name="ps_p", bufs=1, space="PSUM")
            ps_p = ps_p_cm.__enter__()
            kacc = {
                sc4: ps_p.tile([P, NQ], F32, tag=f"ka{sc4}", name=f"ka{sc4}")
                for sc4 in range(4)
            }
            for dt in range(KT_D):
                ek = load_enc("k", dt, "act" if dt % 2 else "dve")
                for sc4 in range(4):
                    nc.tensor.matmul(
                        kacc[sc4][:],
                        wtiles["wk"][:, dt, :],
                        ek[:, sc4 * NQ : (sc4 + 1) * NQ],
                        start=(dt == 0),
                        stop=(dt == KT_D - 1),
                    )
            for sc4 in range(4):
                nc.vector.tensor_copy(
                    kt_sb[:, sc4 * NQ : (sc4 + 1) * NQ], kacc[sc4][:]
                )
            qacc = {
                nn: ps_p.tile([P, NQ], F32, tag=f"qa{nn}", name=f"qa{nn}")
                for nn in range(2)
            }
            for dt in range(KT_D):
                eq = load_enc(
                    "q", dt, "dve" if dt % 2 else "act", cols=(0, SCH),
                    tagsuf="h",
                )
                for nn in range(2):
                    nc.tensor.matmul(
                        qacc[nn][:],
                        wtiles["wq"][:, dt, :],
                        eq[:, nn * NQ : (nn + 1) * NQ],
                        start=(dt == 0),
                        stop=(dt == KT_D - 1),
                    )
            for nn in range(2):
                nc.vector.tensor_copy(
                    qt_sb[:, nn * NQ : (nn + 1) * NQ], qacc[nn][:]
                )
            ps_p_cm.__exit__(None, None, None)

            # ---- attention + late-streamed V / Q-half1 ----
            ps_at_cm = tc.tile_pool(name="ps_at", bufs=1, space="PSUM")
            ps_at = ps_at_cm.__enter__()
            ctx_ps = {}

            def mega_tile():
                return ps_at.tile([P, SCH], F32, tag="mega", bufs=2, name="mega")

            def scores_tt(sc, tt):
                """Emit scores+exp for one t-tile; returns 2 ex tiles."""
                exs = []
                for half in range(SCH // NQ):
                    m = mega_tile()
                    s0 = sc * SCH + half * NQ
                    for h in range(HPC):
                        nc.tensor.matmul(
                            m[:, h * NQ : (h + 1) * NQ],
                            kt_sb[h * DK : (h + 1) * DK, tt * P : (tt + 1) * P],
                            qt_sb[h * DK : (h + 1) * DK, s0 : s0 + NQ],
                            start=True,
                            stop=True,
                        )
                    ex = expp.tile([P, SCH], BF16, tag=f"exp{half}", name="ex")
                    nc.scalar.activation(ex[:], m[:], EXPF, scale=1.0 / np.sqrt(DK))
                    exs.append(ex)
                return exs

            def ctx_tt(sc, tt, exs):
                for h in range(HPC):
                    for half, ex in enumerate(exs):
                        nc.tensor.matmul(
                            ctx_ps[(sc, h)][:, half * NQ : (half + 1) * NQ],
                            v_aug[:, tt, h * (DV + 1) : (h + 1) * (DV + 1)],
                            ex[:, h * NQ : (h + 1) * NQ],
                            start=(tt == 0),
                            stop=(tt == TT - 1),
                        )

            def v_partial(dt):
                """VT partial for one d-tile: transient mega slots + SBUF acc."""
                ev = load_enc("v", dt, "dve")
                for half in range(2):
                    m = mega_tile()
                    for nn in range(2):
                        nc.tensor.matmul(
                            m[:, nn * NQ : (nn + 1) * NQ],
                            wtiles["wv"][:, dt, :],
                            ev[:, half * SCH + nn * NQ : half * SCH + (nn + 1) * NQ],
                            start=True,
                            stop=True,
                        )
                    dst = vt_acc[:, half * SCH : (half + 1) * SCH]
                    if dt == 0:
                        nc.vector.tensor_copy(dst, m[:])
                    else:
                        nc.vector.tensor_add(dst, dst, m[:])

            def q_partial(dt):
                eq = load_enc("q", dt, "dve", cols=(SCH, S), tagsuf="h")
                m = mega_tile()
                for nn in range(2):
                    nc.tensor.matmul(
                        m[:, nn * NQ : (nn + 1) * NQ],
                        wtiles["wq"][:, dt, :],
                        eq[:, nn * NQ : (nn + 1) * NQ],
                        start=True,
                        stop=True,
                    )
                if dt == 0:
                    nc.vector.tensor_copy(qt_acc[:], m[:])
                else:
                    nc.vector.tensor_add(qt_acc[:], qt_acc[:], m[:])

            def v_finish():
                # cast VT f32 -> bf16, PE-transpose into V_aug
                for nn in range(4):
                    nc.vector.tensor_copy(
                        vt_sb[:, nn * NQ : (nn + 1) * NQ],
                        vt_acc[:, nn * NQ : (nn + 1) * NQ],
                    )
                for tt in range(TT):
                    tp = ps_at.tile([P, P], BF16, tag="mega", bufs=2, name="tp")
                    nc.tensor.transpose(
                        tp[:], vt_sb[:, tt * P : (tt + 1) * P], ident[:]
                    )
                    nc.vector.tensor_copy(v_aug[:, tt, 0:DV], tp[:, 0:DV])
                    nc.vector.tensor_copy(
                        v_aug[:, tt, DV + 1 : 2 * DV + 1], tp[:, DV : 2 * DV]
                    )

            def normalize(sc):
                for h in range(HPC):
                    den = misc.tile([1, SCH], F32, tag="den", name="den")
                    nc.vector.tensor_copy(den[:], ctx_ps[(sc, h)][DV : DV + 1, :])
                    recip = misc.tile([1, SCH], F32, tag="recip", name="recip")
                    nc.vector.reciprocal_approx_fast(recip[:], den[:])
                    bcast = misc.tile([DV, SCH], F32, tag="bcast", name="bcast")
                    nc.gpsimd.partition_broadcast(bcast[:], recip[:])
                    nc.vector.tensor_mul(
                        cat_loc[h * DV : (h + 1) * DV, sc * SCH : (sc + 1) * SCH],
                        ctx_ps[(sc, h)][0:DV, :],
                        bcast[:],
                    )
                cb = dram.tile([P, SCH], BF16, tag=f"catb{sc}", name="cb")
                nc.sync.dma_start(cb[:], cat_loc[:, sc * SCH : (sc + 1) * SCH])
                ga = dram.tile([D, SCH], BF16, tag=f"catall{sc}", name="ga")
                nc.gpsimd.collective_compute(
                    "AllGather",
                    mybir.AluOpType.bypass,
                    ins=[cb[:].opt()],
                    outs=[ga[:].opt()],
                    replica_groups=rg,
                )
                return ga

            def outproj(sc, ga):
                m = mega_tile()  # [128, 1024] = two 512-accumulators
                for kt in range(KT_D):
                    ct = catin.tile([P, SCH], BF16, tag="catkt", name="ct")
                    nc.sync.dma_start(ct[:], ga[kt * P : (kt + 1) * P, :])
                    for nn in range(2):
                        nc.tensor.matmul(
                            m[:, nn * NQ : (nn + 1) * NQ],
                            wtiles["wo"][:, kt, :],
                            ct[:, nn * NQ : (nn + 1) * NQ],
                            start=(kt == 0),
                            stop=(kt == KT_D - 1),
                        )
                off = sc * SCH
                nc.vector.tensor_copy(out_sb[:, off : off + SCH], m[:])
                nc.sync.dma_start(
                    out_t[:, off : off + SCH], out_sb[:, off : off + SCH]
                )

            # --- emit: chunk 0 scores with V/Q-h1 streamed between tiles ---
            for h in range(HPC):
                ctx_ps[(0, h)] = ps_at.tile(
                    [DV + 1, SCH], F32, tag=f"ctx{h}", name=f"ctx{h}"
                )
            sc0_exs = []
            for tt in range(TT):
                sc0_exs.append(scores_tt(0, tt))
                if tt < 8:
                    v_partial(tt)
                elif tt == 8:
                    v_finish()
                elif tt < 15:
                    q_partial(tt - 9)  # dt 0..5
            for dt in (6, 7):
                q_partial(dt)
            for nn in range(2):
                nc.vector.tensor_copy(
                    qt_sb[:, SCH + nn * NQ : SCH + (nn + 1) * NQ],
                    qt_acc[:, nn * NQ : (nn + 1) * NQ],
                )

            # --- chunk 1 scores + ctx(0) + ctx(1) interleaved ---
            sc1_exs = []
            for tt in range(TT):
                sc1_exs.append(scores_tt(1, tt))
                ctx_tt(0, tt, sc0_exs[tt])
            ga0 = normalize(0)
            for h in range(HPC):
                ctx_ps[(1, h)] = ps_at.tile(
                    [DV + 1, SCH], F32, tag=f"ctx{h}", name=f"c1{h}"
                )
            for tt in range(TT):
                ctx_tt(1, tt, sc1_exs[tt])
            ga1 = normalize(1)
            outproj(0, ga0)
            outproj(1, ga1)
            ps_at_cm.__exit__(None, None, None)

    nc.compile()
    return nc


def kernel(
    encodings_for_q,
    encodings_for_k,
    encodings_for_v,
    W_q,
    W_k,
    W_v,
    W_out,
    _trace: bool = False,
):
    encodings_for_q = np.asarray(encodings_for_q, dtype=np.float32)
    encodings_for_k = np.asarray(encodings_for_k, dtype=np.float32)
    encodings_for_v = np.asarray(encodings_for_v, dtype=np.float32)
    W_q = np.asarray(W_q, dtype=np.float32)
    W_k = np.asarray(W_k, dtype=np.float32)
    W_v = np.asarray(W_v, dtype=np.float32)
    W_out = np.asarray(W_out, dtype=np.float32)

    if "nc" not in _cache:
        _cache["nc"] = build()
    nc = _cache["nc"]

    eqT = np.ascontiguousarray(encodings_for_q.T)
    ekT = np.ascontiguousarray(encodings_for_k.T)
    evT = np.ascontiguousarray(encodings_for_v.T)

    in_maps = []
    for c in range(NCORES):
        hs = slice(HPC * c, HPC * (c + 1))
        in_maps.append(
            {
                "encq_t": eqT,
                "enck_t": ekT,
                "encv_t": evT,
                "wq": np.ascontiguousarray(
                    np.transpose(W_q[hs], (1, 0, 2)).reshape(D, FW)
                ),
                "wk": np.ascontiguousarray(
                    np.transpose(W_k[hs], (1, 0, 2)).reshape(D, FW)
                ),
                "wv": np.ascontiguousarray(
                    np.transpose(W_v[hs], (1, 0, 2)).reshape(D, FW)
                ),
                "wo": np.ascontiguousarray(W_out[:, FW * c : FW * (c + 1)]),
            }
        )

    r = run_bass_kernel_spmd(
        nc, in_maps, core_ids=list(range(NCORES)), trace=_trace
    )
    out = np.concatenate(
        [r.results[c]["outT"].T for c in range(NCORES)], axis=1
    )
    if _trace:
        kernel.last_exec_time_ns = r.exec_time_ns
        kernel.last_insts = (
            r.instructions_and_trace[0] if r.instructions_and_trace else None
        )
    return out.astype(np.float32)
